# revision 1
# baseline (speedup 1.0000x reference)
"""Trainium2 Bass kernel for the 200-layer 1-channel Conv1d(k=7) chain + FC + sigmoid.

Strategy (pure data parallel, 8 cores, batch 1024 -> 128/core):
  - On-chip layout keeps the sequence dim on SBUF partitions, interleaved mod 128:
      H[p, 128*c + b] = h[b, 128*c + p]
    so each conv layer is a banded matmul contracting over partitions.
  - The 7-tap band is expressed per layer as a [64, 32] stacked weight block
    (D = within-32-group band, E = cross-group band).  Each 512-wide PSUM block
    is computed by 5 concurrent PE matmuls on disjoint 32x32 sub-array tiles:
      3x (K=64 combined D+E), 1x (K=32 D at row-group 3), 1x (K=6 column-wrap).
  - float32r matmuls (full PE rate at N>=256); relu+bias applied on the
    PSUM->SBUF copy, alternating between the Scalar (ACT) and Vector (DVE)
    engines so neither becomes the serial bottleneck.
  - x is DMA'd naturally and transposed on-chip through the PE (f32).
  - Final Linear(188->91) runs as two accumulating matmuls + fused Sigmoid.
"""

import os
import sys

if "/opt/trn_rl_repo" not in sys.path:
    sys.path.insert(0, "/opt/trn_rl_repo")

from contextlib import ExitStack

import numpy as np

import concourse.bacc as bacc
import concourse.bass as bass
import concourse.mybir as mybir
from concourse import tile
from concourse.bass_utils import run_bass_kernel_spmd

N_CORES = 8
BC = 128          # batch per core
L0 = 1388
N_LAYERS = 200
K7 = 7
FC_IN = 188
FC_OUT = 91

F32 = mybir.dt.float32
F32R = mybir.dt.float32r
BF16 = mybir.dt.bfloat16
AFT = mybir.ActivationFunctionType
ALU = mybir.AluOpType

MODE = "packed32"         # "packed64" | "packed32" | "simple"
BAND_CHUNKS = 4           # weight DMA prefetch chunks (50 layers each)


def _make_bands(conv_w: np.ndarray) -> np.ndarray:
    """[128, 200*64] f32.  Layer l occupies free cols [64l, 64l+64):
    cols [64l, 64l+32)  = D (D[j,r]=w[j-r]) replicated in all four
                          32-partition groups (weights must share the rhs
                          base partition);
    cols [64l+32, 64l+64) = E (E[j,r]=w[32+j-r], rows 0:6) replicated at
                          partition bases 0/32/64/96."""
    out = np.zeros((128, N_LAYERS * 64), np.float32)
    j = np.arange(32)[:, None]
    r = np.arange(32)[None, :]
    dd = j - r            # D taps at 0..6
    ee = 32 + j - r       # E taps at 0..6
    for l in range(N_LAYERS):
        w = conv_w[l]
        D = np.where((dd >= 0) & (dd <= 6), w[np.clip(dd, 0, 6)], 0.0)
        E = np.where((ee >= 0) & (ee <= 6), w[np.clip(ee, 0, 6)], 0.0)
        fo = 64 * l
        for g in range(4):
            out[32 * g:32 * g + 32, fo:fo + 32] = D
            out[32 * g:32 * g + 6, fo + 32:fo + 64] = E[0:6]
    return out


def _make_bands_full(conv_w: np.ndarray) -> np.ndarray:
    """[128, 200*128] f32: per layer the full 128x128 within-column band."""
    out = np.zeros((128, N_LAYERS * 128), np.float32)
    j = np.arange(128)[:, None]
    r = np.arange(128)[None, :]
    dd = j - r
    for l in range(N_LAYERS):
        w = conv_w[l]
        out[:, 128 * l:128 * (l + 1)] = np.where(
            (dd >= 0) & (dd <= 6), w[np.clip(dd, 0, 6)], 0.0)
    return out




def _make_bands2(conv_w: np.ndarray) -> np.ndarray:
    """[128, 200*256]: cols [256l,256l+128) full within-column band
    (B[j,r]=w[j-r], 0<=j-r<=6); cols [256l+128,256l+256) rows 0:6 wrap
    (W[j,r]=w[128+j-r], nonzero r>=122)."""
    out = np.zeros((128, N_LAYERS * 256), np.float32)
    j = np.arange(128)[:, None]
    r = np.arange(128)[None, :]
    dd = j - r
    j6 = np.arange(6)[:, None]
    ww = 128 + j6 - r
    for l in range(N_LAYERS):
        w = conv_w[l]
        fo = 256 * l
        out[:, fo:fo + 128] = np.where((dd >= 0) & (dd <= 6),
                                       w[np.clip(dd, 0, 6)], 0.0)
        out[0:6, fo + 128:fo + 256] = np.where((ww >= 0) & (ww <= 6),
                                               w[np.clip(ww, 0, 6)], 0.0)
    return out

def _ceil_div(a, b):
    return -(-a // b)


def build_program(conv_b: np.ndarray, mode: str = MODE):
    """Build + schedule the Tile program.  Returns the Bacc object."""
    nc = bacc.Bacc("TRN2", target_bir_lowering=False, debug=False,
                   enable_asserts=True)

    x_d = nc.dram_tensor("xs", [BC, L0], F32, kind="ExternalInput").ap()
    cb_d = nc.dram_tensor("cb", [128, N_LAYERS], F32, kind="ExternalInput").ap()
    fcw_d = nc.dram_tensor("fcw", [FC_IN, FC_OUT], F32, kind="ExternalInput").ap()
    fcb_d = nc.dram_tensor("fcb", [FC_OUT, 1], F32, kind="ExternalInput").ap()
    id_d = nc.dram_tensor("ident", [128, 128], F32, kind="ExternalInput").ap()
    if mode == "simple":
        bands_d = nc.dram_tensor("bandsf", [128, N_LAYERS * 128], F32,
                                 kind="ExternalInput").ap()
    else:
        bands_d = nc.dram_tensor("bands", [128, N_LAYERS * 256], BF16,
                                 kind="ExternalInput").ap()
    smoke = int(os.environ.get("KERNEL_SMOKE_LAYERS", "0"))
    nle = smoke if smoke > 0 else N_LAYERS
    if smoke:
        out_d = nc.dram_tensor("outT", [128, 256], F32,
                               kind="ExternalOutput").ap()
    else:
        out_d = nc.dram_tensor("outT", [FC_OUT, BC], F32,
                               kind="ExternalOutput").ap()

    nc0 = _ceil_div(L0, 128)  # 11

    with tile.TileContext(nc) as tc, ExitStack() as ctx:
        cpool = ctx.enter_context(tc.tile_pool(name="const", bufs=1))
        hpool = ctx.enter_context(tc.tile_pool(name="h", bufs=3))
        pspool = ctx.enter_context(tc.tile_pool(name="ps", bufs=6, space="PSUM"))

        xs = cpool.tile([128, L0], F32, tag="xs")
        ident = cpool.tile([128, 128], F32, tag="ident")
        cb_s = cpool.tile([128, N_LAYERS], F32, tag="cb")
        fca = cpool.tile([128, FC_OUT], F32, tag="fca")
        fcb60 = cpool.tile([FC_IN - 128, FC_OUT], F32, tag="fcb60")
        fcbias = cpool.tile([FC_OUT, 1], F32, tag="fcbias")
        outs = cpool.tile([FC_OUT, BC], F32, tag="outs")

        nc.sync.dma_start(xs[:, :], x_d[:, :])
        nc.sync.dma_start(ident[:, :], id_d[:, :])
        nc.sync.dma_start(cb_s[:, :], cb_d[:, :])
        nc.sync.dma_start(fca[:, :], fcw_d[0:128, :])
        nc.sync.dma_start(fcb60[:, :], fcw_d[128:FC_IN, :])
        nc.sync.dma_start(fcbias[:, :], fcb_d[:, :])

        layers_per_chunk = _ceil_div(N_LAYERS, BAND_CHUNKS)
        bw = layers_per_chunk * 256
        bchunks = []
        for i in range(BAND_CHUNKS):
            t = cpool.tile([128, bw], BF16, tag=f"bands{i}")
            nc.sync.dma_start(t[:, :], bands_d[:, bw * i:bw * (i + 1)])
            bchunks.append(t)

        # ---- load x and transpose into H0 (L on partitions) ----
        h0 = hpool.tile([128, nc0 * 128], BF16, tag="h")
        nc.gpsimd.memset(h0[:, 128 * (nc0 - 1):], 0.0)  # finite tail
        cnt = 0
        for c in range(nc0):
            w = min(128, L0 - 128 * c)
            pt = pspool.tile([128, 128], F32, tag="ps")
            nc.tensor.transpose(pt[0:w, :], xs[:, 128 * c:128 * c + w],
                                ident[:, :])
            if cnt % 2 == 0:
                nc.scalar.activation(h0[0:w, 128 * c:128 * c + 128],
                                     pt[0:w, :], AFT.Identity, bias=0.0,
                                     scale=1.0)
            else:
                nc.vector.tensor_copy(h0[0:w, 128 * c:128 * c + 128],
                                      pt[0:w, :])
            cnt += 1

        # ---- 200 conv layers ----
        hin = h0
        Lc = L0
        for l in range(nle):
            Lo = Lc - 6
            nci = _ceil_div(Lc, 128)
            nco = _ceil_div(Lo, 128)
            hout = hpool.tile([128, nco * 128],
                              BF16 if l < N_LAYERS - 1 else F32, tag="h")

            if mode == "simple":
                ch = bchunks[l // layers_per_chunk]
                fo = 128 * (l % layers_per_chunk)
                stF = ch[0:128, fo:fo + 128]
                stW = None  # wrap handled from stacked layout only
                # simple mode still needs the E corner for the wrap matmul:
                # reuse the full band's top-right 6x32 corner? Not available.
                # -> simple mode computes the wrap from the full band's
                #    rows 0:6, cols 96:128 equivalent is NOT present there,
                #    so build the wrap lhsT from the D band layout: the wrap
                #    weights equal w[32+j-r] on rows 0:6, cols 26:32 of a
                #    32-block; the same values appear in the full band at
                #    [j, r] = [j, 96+rr] with j-(96+rr) = ... not present.
                raise NotImplementedError(
                    "simple mode needs the stacked bands input as well")
            else:
                ch = bchunks[l // layers_per_chunk]
                fo = 256 * (l % layers_per_chunk)
                bandf = ch[:, fo:fo + 128]
                wrapf = ch[0:6, fo + 128:fo + 256]

            bval = float(conv_b[l])
            c0 = 0
            while c0 < nco:
                c1 = min(c0 + 4, nco)
                N = (c1 - c0) * 128
                q0 = 128 * c0
                ps = pspool.tile([128, N], F32, tag="ps")
                wn = (min(c1, nci - 1) - c0) * 128

                nc.tensor.matmul(ps[:, 0:N], bandf,
                                 hin[:, q0:q0 + N],
                                 start=True, stop=(wn <= 0),
                                 skip_group_check=True)
                if wn > 0:
                    nc.tensor.matmul(ps[:, 0:wn], wrapf,
                                     hin[0:6, q0 + 128:q0 + 128 + wn],
                                     start=False, stop=True,
                                     skip_group_check=True)

                if l < nle - 1 or smoke:
                    if cnt % 2 == 0:
                        nc.scalar.activation(hout[:, q0:q0 + N], ps[:, 0:N],
                                             AFT.Relu, bias=cb_s[:, l:l + 1],
                                             scale=1.0)
                    else:
                        nc.vector.tensor_scalar(hout[:, q0:q0 + N],
                                                ps[:, 0:N], bval, 0.0,
                                                op0=ALU.add, op1=ALU.max)
                else:
                    nc.vector.tensor_scalar(hout[:, q0:q0 + N], ps[:, 0:N],
                                            bval, None, op0=ALU.add)
                cnt += 1
                c0 = c1

            hin = hout
            Lc = Lo

        if smoke:
            sout = cpool.tile([128, 256], F32, tag="souts")
            nc.scalar.activation(sout[:, :], hin[:, 0:256], AFT.Identity,
                                 bias=0.0, scale=1.0)
            nc.sync.dma_start(out_d[:, :], sout[:, :])
        else:
            # ---- FC 188 -> 91 + sigmoid ----
            assert Lc == FC_IN
            fps = pspool.tile([FC_OUT, BC], F32, tag="ps")
            nc.tensor.matmul(fps[:, :], fca[0:128, :],
                             hin[:, 0:128], start=True, stop=False)
            nc.tensor.matmul(fps[:, :], fcb60[0:FC_IN - 128, :],
                             hin[0:FC_IN - 128, 128:256],
                             start=False, stop=True)
            nc.scalar.activation(outs[:, :], fps[:, :], AFT.Sigmoid,
                                 bias=fcbias[0:FC_OUT, 0:1], scale=1.0)
            nc.sync.dma_start(out_d[:, :], outs[:, :])

    nc.compile()
    return nc


def make_in_maps(x, conv_w, conv_b, fc_w, fc_b, mode: str = MODE):
    x = np.ascontiguousarray(x, np.float32)
    import ml_dtypes
    bands = np.ascontiguousarray(
        _make_bands2(np.asarray(conv_w, np.float32)).astype(
            ml_dtypes.bfloat16))
    cb = np.ascontiguousarray(
        np.broadcast_to(np.asarray(conv_b, np.float32)[None, :],
                        (128, N_LAYERS)))
    fcw = np.ascontiguousarray(np.asarray(fc_w, np.float32).T)  # [188, 91]
    fcb = np.ascontiguousarray(np.asarray(fc_b, np.float32)[:, None])
    ident = np.eye(128, dtype=np.float32)
    bname = "bandsf" if mode == "simple" else "bands"
    in_maps = []
    for i in range(N_CORES):
        in_maps.append({
            "xs": np.ascontiguousarray(x[BC * i:BC * (i + 1)]),
            bname: bands,
            "cb": cb,
            "fcw": fcw,
            "fcb": fcb,
            "ident": ident,
        })
    return in_maps


def run(x, conv_w, conv_b, fc_w, fc_b, mode: str = MODE, **spmd_kwargs):
    nc = build_program(np.asarray(conv_b, np.float32), mode)
    in_maps = make_in_maps(x, conv_w, conv_b, fc_w, fc_b, mode)
    res = run_bass_kernel_spmd(nc, in_maps, list(range(N_CORES)),
                               **spmd_kwargs)
    out = np.concatenate([r["outT"].T for r in res.results], axis=0)
    return np.ascontiguousarray(out, np.float32), res


def kernel(x, conv_w, conv_b, fc_w, fc_b):
    out, _ = run(x, conv_w, conv_b, fc_w, fc_b)
    return out


if __name__ == "__main__":
    rng = np.random.default_rng(0)
    x = rng.normal(size=(1024, L0)).astype(np.float32)
    s = 1.0 / np.sqrt(K7)
    cw = rng.uniform(-s, s, (N_LAYERS, K7)).astype(np.float32)
    cb = rng.uniform(-s, s, N_LAYERS).astype(np.float32)
    sf = 1.0 / np.sqrt(FC_IN)
    fw = rng.uniform(-sf, sf, (FC_OUT, FC_IN)).astype(np.float32)
    fb = rng.uniform(-sf, sf, FC_OUT).astype(np.float32)
    out = kernel(x, cw, cb, fw, fb)
    print(out.shape, out.dtype)



# revision 8
# speedup vs baseline: 1.4399x; 1.4399x over previous
"""Trainium2 Bass kernel: 200-layer 1-channel Conv1d(k=7) chain + FC + sigmoid.

Strategy (pure data parallel, 8 cores, batch 1024 -> 128/core):
  - Sequence dim on SBUF partitions, disjoint 128-position chunks;
    free dim = (chunk, batch).  Batch 128 split into two streams of 64
    so engine latency (matmul -> psum copy -> next matmul) overlaps
    across streams.
  - Each conv layer is ONE fp8 DoubleRow matmul pass per 8-chunk group:
    k-tile 0 = in-chunk band (B[j,r] = w[j-r]), k-tile 1 = cross-chunk
    wrap (rows 0:6), contracted together by the PE's 128x256 virtual
    array at 0.5 cycles/column.  No separate wrap matmuls.
  - PSUM->SBUF relu+bias copies alternate between Scalar (ACT) and
    Vector (DVE) engines via a greedy load-balance; h is stored fp8e4
    (error washes out: conv gain sqrt(sum w^2)=0.577/layer).
  - Final Linear(188->91) is one DoubleRow matmul per stream
    (k-tiles = the two position chunks) + fused Sigmoid.
"""

import sys

if "/opt/trn_rl_repo" not in sys.path:
    sys.path.insert(0, "/opt/trn_rl_repo")

from contextlib import ExitStack

import numpy as np
import ml_dtypes

import concourse.bacc as bacc
import concourse.mybir as mybir
from concourse import tile
from concourse.bass_utils import run_bass_kernel_spmd

N_CORES = 8
BC = 128          # batch per core
L0 = 1388
N_LAYERS = 200
FC_IN = 188
FC_OUT = 91

F32 = mybir.dt.float32
BF16 = mybir.dt.bfloat16
F8 = mybir.dt.float8e4
E4M3 = ml_dtypes.float8_e4m3
AFT = mybir.ActivationFunctionType
ALU = mybir.AluOpType
DR = mybir.MatmulPerfMode.DoubleRow

W_S = 64          # stream width (batch cols per stream); must be %16
CPG = 8           # chunks per matmul group (8*64 = 512 psum cols)
WCHUNK_LAYERS = 10  # layers per weight-DMA chunk


def _ceil_div(a, b):
    return -(-a // b)


def _make_bands(conv_w: np.ndarray) -> np.ndarray:
    """[128, 200*256] f32: cols [256l,256l+128) in-chunk band
    (B[j,r]=w[j-r], 0<=j-r<=6); cols [256l+128,256l+256) rows 0:6
    cross-chunk wrap (W[j,r]=w[128+j-r], nonzero r>=122)."""
    out = np.zeros((128, N_LAYERS * 256), np.float32)
    j = np.arange(128)[:, None]
    r = np.arange(128)[None, :]
    dd = j - r
    j6 = np.arange(6)[:, None]
    ww = 128 + j6 - r
    for l in range(N_LAYERS):
        w = conv_w[l]
        fo = 256 * l
        out[:, fo:fo + 128] = np.where((dd >= 0) & (dd <= 6),
                                       w[np.clip(dd, 0, 6)], 0.0)
        out[0:6, fo + 128:fo + 256] = np.where((ww >= 0) & (ww <= 6),
                                               w[np.clip(ww, 0, 6)], 0.0)
    return out


def _make_fcpack(fc_w: np.ndarray) -> np.ndarray:
    """[128, 192]: k-tile 0 = fc_w.T rows 0:128 (91 cols pad to 96),
    k-tile 1 = fc_w.T rows 128:188 zero-padded to 128 rows."""
    fcwT = np.asarray(fc_w, np.float32).T  # [188, 91]
    out = np.zeros((128, 192), np.float32)
    out[:, 0:FC_OUT] = fcwT[0:128]
    out[0:FC_IN - 128, 96:96 + FC_OUT] = fcwT[128:FC_IN]
    return out


def _dr_view(sl, kstep, n):
    """[P, W] slice -> [P, 2, n] DoubleRow view (k-tile step = kstep)."""
    c = sl.copy()
    pstep, pcount = c.ap[0]
    c.ap = type(c.ap)([[pstep, pcount], [kstep, 2], [1, n]])
    return c


class _CopyBalancer:
    """Greedy assignment of PSUM->SBUF copies to ACT/DVE by busy time."""

    def __init__(self, nc):
        self.nc = nc
        self.busy = {"act": 0.0, "dve": 0.0}

    def copy(self, dst, src, ncols, bias_ap, bias_val, relu):
        ca = ncols / 1.2 + 143.0
        cd = ncols / 0.96 + 125.0
        if self.busy["act"] + ca <= self.busy["dve"] + cd:
            eng, cost = "act", ca
        else:
            eng, cost = "dve", cd
        self.busy[eng] += cost
        if eng == "act":
            if relu:
                self.nc.scalar.activation(dst, src, AFT.Relu,
                                          bias=bias_ap, scale=1.0)
            else:
                self.nc.scalar.activation(dst, src, AFT.Identity,
                                          bias=bias_ap, scale=1.0)
        else:
            if relu:
                self.nc.vector.tensor_scalar(dst, src, bias_val, 0.0,
                                             op0=ALU.add, op1=ALU.max)
            else:
                self.nc.vector.tensor_scalar(dst, src, bias_val, None,
                                             op0=ALU.add)


def build_program(conv_b: np.ndarray):
    nc = bacc.Bacc("TRN2", target_bir_lowering=False, debug=False,
                   enable_asserts=True)

    x_d = nc.dram_tensor("xs", [BC, L0], F32, kind="ExternalInput").ap()
    cb_d = nc.dram_tensor("cb", [128, N_LAYERS], F32,
                          kind="ExternalInput").ap()
    bands_d = nc.dram_tensor("bands", [128, N_LAYERS * 256], F8,
                             kind="ExternalInput").ap()
    fcp_d = nc.dram_tensor("fcpack", [128, 192], F8,
                           kind="ExternalInput").ap()
    fcb_d = nc.dram_tensor("fcb", [FC_OUT, 1], F32, kind="ExternalInput").ap()
    id_d = nc.dram_tensor("ident", [128, 128], F32,
                          kind="ExternalInput").ap()
    out_d = nc.dram_tensor("outT", [FC_OUT, BC], F32,
                           kind="ExternalOutput").ap()

    nchunk0 = _ceil_div(L0, 128)   # 11 input chunks

    with tile.TileContext(nc) as tc, ExitStack() as ctx:
        cpool = ctx.enter_context(tc.tile_pool(name="const", bufs=1))
        hpool = ctx.enter_context(tc.tile_pool(name="h", bufs=3))
        pspool = ctx.enter_context(tc.tile_pool(name="ps", bufs=2,
                                                space="PSUM"))

        xs = cpool.tile([BC, L0], F32, tag="xs")
        ident = cpool.tile([128, 128], F32, tag="ident")
        cb_s = cpool.tile([128, N_LAYERS], F32, tag="cb")
        fcp = cpool.tile([128, 192], F8, tag="fcp")
        fcbias = cpool.tile([FC_OUT, 1], F32, tag="fcbias")
        outs = cpool.tile([FC_OUT, BC], F32, tag="outs")

        nc.sync.dma_start(xs[:, :], x_d[:, :])
        nc.sync.dma_start(ident[:, :], id_d[:, :])
        nc.sync.dma_start(cb_s[:, :], cb_d[:, :])
        nc.sync.dma_start(fcp[:, :], fcp_d[:, :])
        nc.sync.dma_start(fcbias[:, :], fcb_d[:, :])

        nwc = _ceil_div(N_LAYERS, WCHUNK_LAYERS)
        bw = WCHUNK_LAYERS * 256
        bchunks = []
        for i in range(nwc):
            t = cpool.tile([128, bw], F8, tag=f"bands{i}")
            nc.sync.dma_start(t[:, :], bands_d[:, bw * i:bw * (i + 1)])
            bchunks.append(t)

        bal = _CopyBalancer(nc)

        # ---- input: transpose via PE (f32), split into the two streams ----
        h0 = [hpool.tile([128, (nchunk0 + 1) * W_S], F8, tag=f"h{s}",
                         name=f"h0_{s}")
              for s in range(2)]
        for s in range(2):
            nc.gpsimd.memset(h0[s][:, :], 0.0)

        tcnt = 0
        c = 0
        while c < nchunk0:
            cn = min(c + 4, nchunk0)
            pt = pspool.tile([128, 512], F32, tag=f"ps{tcnt % 2}",
                             name=f"pt{tcnt}")
            tcnt += 1
            for ci in range(c, cn):
                w = min(128, L0 - 128 * ci)
                fo2 = 128 * (ci - c)
                if w < 128:
                    # partial chunk: clear the never-written psum rows so
                    # no stale inf/NaN reaches the fp8 h tiles (32-aligned
                    # partition base; transpose rewrites the valid overlap)
                    base = (w // 32) * 32
                    nc.vector.memset(pt[base:128, fo2:fo2 + 128], 0.0)
                nc.tensor.transpose(pt[0:w, fo2:fo2 + 128],
                                    xs[:, 128 * ci:128 * ci + w],
                                    ident[:, :])
            # split copies into the two stream tiles (3D strided APs)
            nb = cn - c
            for s in range(2):
                src = pt[:, 64 * s:64 * s + (nb - 1) * 128 + 64]
                sv = src.copy()
                pstep = sv.ap[0][0]
                sv.ap = type(sv.ap)([[pstep, 128], [128, nb], [1, 64]])
                dst = h0[s][:, W_S * c: W_S * cn]
                dv = dst.copy()
                pstep2 = dv.ap[0][0]
                dv.ap = type(dv.ap)([[pstep2, 128], [64, nb], [1, 64]])
                bal.copy(dv, sv, nb * 64, 0.0, 0.0, False)
            c = cn

        # ---- 200 conv layers ----
        hin = h0
        for j in range(1, N_LAYERS + 1):
            Lo = L0 - 6 * j
            nco = _ceil_div(Lo, 128)
            G = _ceil_div(nco, CPG)
            lw = bchunks[(j - 1) // WCHUNK_LAYERS]
            fo = 256 * ((j - 1) % WCHUNK_LAYERS)
            lhsT = lw[:, fo:fo + 256].rearrange("p (two m) -> p two m", two=2)

            bias_ap = cb_s[:, j - 1:j]
            bias_val = float(conv_b[j - 1])
            relu = j < N_LAYERS

            hout = [hpool.tile([128, (nco + 1) * W_S], F8, tag=f"h{s}",
                               name=f"h{j}_{s}")
                    for s in range(2)]
            if j <= 2:
                # first tenancy of pool buffers: clear virgin SBUF so the
                # pad/garbage regions stay finite fp8 forever
                for s in range(2):
                    nc.gpsimd.memset(hout[s][:, :], 0.0)

            pss = []
            for s in range(2):
                ps = pspool.tile([128, G * 512], F32, tag=f"ps{s}",
                                 name=f"ps{j}_{s}")
                for g in range(G):
                    c0 = CPG * g
                    c1 = min(c0 + CPG, nco)
                    N = (c1 - c0) * W_S
                    rhs = _dr_view(hin[s][:, c0 * W_S:c0 * W_S + N + W_S],
                                   W_S, N)
                    nc.tensor.matmul(ps[:, 512 * g:512 * g + N], lhsT, rhs,
                                     start=True, stop=True, perf_mode=DR,
                                     skip_group_check=True)
                pss.append(ps)

            for s in range(2):
                for g in range(G):
                    c0 = CPG * g
                    c1 = min(c0 + CPG, nco)
                    N = (c1 - c0) * W_S
                    bal.copy(hout[s][:, c0 * W_S:c0 * W_S + N],
                             pss[s][:, 512 * g:512 * g + N],
                             N, bias_ap, bias_val, relu)

            hin = hout

        # ---- FC 188 -> 91 (DoubleRow over the two position chunks) ----
        fclhs = fcp[:, 0:192].rearrange("p (two m) -> p two m", two=2)
        fps = pspool.tile([96, 128], F32, tag="ps0")
        for s in range(2):
            rhs = _dr_view(hin[s][:, 0:128], W_S, W_S)
            nc.tensor.matmul(fps[:, 64 * s:64 * s + 64], fclhs, rhs,
                             start=True, stop=True, perf_mode=DR,
                             skip_group_check=True)
        nc.scalar.activation(outs[:, :], fps[0:FC_OUT, :], AFT.Sigmoid,
                             bias=fcbias[0:FC_OUT, 0:1], scale=1.0)
        nc.sync.dma_start(out_d[:, :], outs[:, :])

    nc.compile()
    return nc


def make_in_maps(x, conv_w, conv_b, fc_w, fc_b):
    x = np.ascontiguousarray(np.asarray(x, np.float32))
    bands = np.ascontiguousarray(
        _make_bands(np.asarray(conv_w, np.float32)).astype(E4M3))
    cb = np.ascontiguousarray(
        np.broadcast_to(np.asarray(conv_b, np.float32)[None, :],
                        (128, N_LAYERS)))
    fcpack = np.ascontiguousarray(_make_fcpack(fc_w).astype(E4M3))
    fcb = np.ascontiguousarray(np.asarray(fc_b, np.float32)[:, None])
    ident = np.eye(128, dtype=np.float32)
    in_maps = []
    for i in range(N_CORES):
        in_maps.append({
            "xs": np.ascontiguousarray(x[BC * i:BC * (i + 1)]),
            "bands": bands,
            "cb": cb,
            "fcpack": fcpack,
            "fcb": fcb,
            "ident": ident,
        })
    return in_maps


def run(x, conv_w, conv_b, fc_w, fc_b, **spmd_kwargs):
    nc = build_program(np.asarray(conv_b, np.float32))
    in_maps = make_in_maps(x, conv_w, conv_b, fc_w, fc_b)
    res = run_bass_kernel_spmd(nc, in_maps, list(range(N_CORES)),
                               **spmd_kwargs)
    out = np.concatenate([r["outT"].T for r in res.results], axis=0)
    return np.ascontiguousarray(out, np.float32), res


def kernel(x, conv_w, conv_b, fc_w, fc_b):
    out, _ = run(x, conv_w, conv_b, fc_w, fc_b)
    return out


if __name__ == "__main__":
    rng = np.random.default_rng(0)
    x = rng.normal(size=(1024, L0)).astype(np.float32)
    s = 1.0 / np.sqrt(7)
    cw = rng.uniform(-s, s, (N_LAYERS, 7)).astype(np.float32)
    cb = rng.uniform(-s, s, N_LAYERS).astype(np.float32)
    sf = 1.0 / np.sqrt(FC_IN)
    fw = rng.uniform(-sf, sf, (FC_OUT, FC_IN)).astype(np.float32)
    fb = rng.uniform(-sf, sf, FC_OUT).astype(np.float32)
    out = kernel(x, cw, cb, fw, fb)
    print(out.shape, out.dtype)


# revision 13
# speedup vs baseline: 2.3413x; 1.6260x over previous
"""Trainium2 Bass kernel: 200-layer 1-channel Conv1d(k=7) chain + FC + sigmoid.

Strategy (pure data parallel, 8 cores, batch 1024 -> 128/core):
  - Sequence dim on SBUF partitions, disjoint 128-position chunks;
    free dim = (chunk, batch).  Batch 128 split into two streams of 64
    so engine latency (matmul -> psum copy -> next matmul) overlaps
    across streams.
  - Each conv layer is ONE fp8 DoubleRow matmul pass per 8-chunk group:
    k-tile 0 = in-chunk band (B[j,r] = w[j-r]), k-tile 1 = cross-chunk
    wrap (rows 0:6), contracted together by the PE's 128x256 virtual
    array at 0.5 cycles/column.  No separate wrap matmuls.
  - PSUM->SBUF relu+bias copies alternate between Scalar (ACT) and
    Vector (DVE) engines via a greedy load-balance; h is stored fp8e4
    (error washes out: conv gain sqrt(sum w^2)=0.577/layer).
  - Final Linear(188->91) is one DoubleRow matmul per stream
    (k-tiles = the two position chunks) + fused Sigmoid.
"""

import sys

if "/opt/trn_rl_repo" not in sys.path:
    sys.path.insert(0, "/opt/trn_rl_repo")

from contextlib import ExitStack

import numpy as np
import ml_dtypes

import concourse.bacc as bacc
import concourse.mybir as mybir
from concourse import tile
from concourse.bass_utils import run_bass_kernel_spmd

N_CORES = 8
BC = 128          # batch per core
L0 = 1388
N_LAYERS = 200
FC_IN = 188
FC_OUT = 91

F32 = mybir.dt.float32
BF16 = mybir.dt.bfloat16
F8 = mybir.dt.float8e4
E4M3 = ml_dtypes.float8_e4m3
AFT = mybir.ActivationFunctionType
ALU = mybir.AluOpType
DR = mybir.MatmulPerfMode.DoubleRow

W_S = 32          # stream width (batch cols per stream); must be %16
N_STR = 4         # batch streams (chain-parallelism); 4 x 32 = 128
WCHUNK_LAYERS = 10  # layers per weight-DMA chunk


def _ceil_div(a, b):
    return -(-a // b)


def _make_bands(conv_w: np.ndarray) -> np.ndarray:
    """[128, 200*256] f32: cols [256l,256l+128) in-chunk band
    (B[j,r]=w[j-r], 0<=j-r<=6); cols [256l+128,256l+256) rows 0:6
    cross-chunk wrap (W[j,r]=w[128+j-r], nonzero r>=122)."""
    out = np.zeros((128, N_LAYERS * 256), np.float32)
    j = np.arange(128)[:, None]
    r = np.arange(128)[None, :]
    dd = j - r
    j6 = np.arange(6)[:, None]
    ww = 128 + j6 - r
    for l in range(N_LAYERS):
        w = conv_w[l]
        fo = 256 * l
        out[:, fo:fo + 128] = np.where((dd >= 0) & (dd <= 6),
                                       w[np.clip(dd, 0, 6)], 0.0)
        out[0:6, fo + 128:fo + 256] = np.where((ww >= 0) & (ww <= 6),
                                               w[np.clip(ww, 0, 6)], 0.0)
    return out


def _make_fcpack(fc_w: np.ndarray) -> np.ndarray:
    """[128, 192]: k-tile 0 = fc_w.T rows 0:128 (91 cols pad to 96),
    k-tile 1 = fc_w.T rows 128:188 zero-padded to 128 rows."""
    fcwT = np.asarray(fc_w, np.float32).T  # [188, 91]
    out = np.zeros((128, 192), np.float32)
    out[:, 0:FC_OUT] = fcwT[0:128]
    out[0:FC_IN - 128, 96:96 + FC_OUT] = fcwT[128:FC_IN]
    return out


def _dr_view(sl, kstep, n):
    """[P, W] slice -> [P, 2, n] DoubleRow view (k-tile step = kstep)."""
    c = sl.copy()
    pstep, pcount = c.ap[0]
    c.ap = type(c.ap)([[pstep, pcount], [kstep, 2], [1, n]])
    return c


class _CopyBalancer:
    """Greedy assignment of PSUM->SBUF copies to ACT/DVE by busy time."""

    def __init__(self, nc):
        self.nc = nc
        self.busy = {"act": 0.0, "dve": 0.0}

    def copy(self, dst, src, ncols, bias_ap, bias_val, relu):
        ca = ncols * 1.06 + 177.0
        cd = ncols * 1.10 + 150.0
        if self.busy["act"] + ca <= self.busy["dve"] + cd:
            eng, cost = "act", ca
        else:
            eng, cost = "dve", cd
        self.busy[eng] += cost
        if eng == "act":
            if relu:
                self.nc.scalar.activation(dst, src, AFT.Relu,
                                          bias=bias_ap, scale=1.0)
            else:
                self.nc.scalar.activation(dst, src, AFT.Identity,
                                          bias=bias_ap, scale=1.0)
        else:
            if relu:
                self.nc.vector.tensor_scalar(dst, src, bias_val, 0.0,
                                             op0=ALU.add, op1=ALU.max)
            else:
                self.nc.vector.tensor_scalar(dst, src, bias_val, None,
                                             op0=ALU.add)


def build_program(conv_b: np.ndarray):
    nc = bacc.Bacc("TRN2", target_bir_lowering=False, debug=False,
                   enable_asserts=True)

    x_d = nc.dram_tensor("xs", [BC, L0], F32, kind="ExternalInput").ap()
    cb_d = nc.dram_tensor("cb", [128, N_LAYERS], F32,
                          kind="ExternalInput").ap()
    bands_d = nc.dram_tensor("bands", [128, N_LAYERS * 256], F8,
                             kind="ExternalInput").ap()
    fcp_d = nc.dram_tensor("fcpack", [128, 192], F8,
                           kind="ExternalInput").ap()
    fcb_d = nc.dram_tensor("fcb", [FC_OUT, 1], F32, kind="ExternalInput").ap()
    id_d = nc.dram_tensor("ident", [128, 128], F32,
                          kind="ExternalInput").ap()
    out_d = nc.dram_tensor("outT", [FC_OUT, BC], F32,
                           kind="ExternalOutput").ap()

    nchunk0 = _ceil_div(L0, 128)   # 11 input chunks

    with tile.TileContext(nc) as tc, ExitStack() as ctx:
        cpool = ctx.enter_context(tc.tile_pool(name="const", bufs=1))
        hpool = ctx.enter_context(tc.tile_pool(name="h", bufs=3))
        pspool = ctx.enter_context(tc.tile_pool(name="ps", bufs=2,
                                                space="PSUM"))

        xs = cpool.tile([BC, L0], F32, tag="xs")
        ident = cpool.tile([128, 128], F32, tag="ident")
        cb_s = cpool.tile([128, N_LAYERS], F32, tag="cb")
        fcp = cpool.tile([128, 192], F8, tag="fcp")
        fcbias = cpool.tile([FC_OUT, 1], F32, tag="fcbias")
        outs = cpool.tile([FC_OUT, BC], F32, tag="outs")

        nc.sync.dma_start(xs[:, :], x_d[:, :])
        nc.sync.dma_start(ident[:, :], id_d[:, :])
        nc.sync.dma_start(cb_s[:, :], cb_d[:, :])
        nc.sync.dma_start(fcp[:, :], fcp_d[:, :])
        nc.sync.dma_start(fcbias[:, :], fcb_d[:, :])

        nwc = _ceil_div(N_LAYERS, WCHUNK_LAYERS)
        bw = WCHUNK_LAYERS * 256
        bchunks = []
        for i in range(nwc):
            t = cpool.tile([128, bw], F8, tag=f"bands{i}")
            nc.sync.dma_start(t[:, :], bands_d[:, bw * i:bw * (i + 1)])
            bchunks.append(t)

        bal = _CopyBalancer(nc)

        # ---- input: transpose via PE (f32), split into the streams ----
        h0 = [hpool.tile([128, (nchunk0 + 1) * W_S], F8, tag=f"h{s}",
                         name=f"h0_{s}")
              for s in range(N_STR)]
        for s in range(N_STR):
            nc.gpsimd.memset(h0[s][:, :], 0.0)

        tcnt = 0
        c = 0
        while c < nchunk0:
            cn = min(c + 4, nchunk0)
            pt = pspool.tile([128, 512], F32, tag=f"ps{tcnt % N_STR}",
                             name=f"pt{tcnt}")
            tcnt += 1
            for ci in range(c, cn):
                w = min(128, L0 - 128 * ci)
                fo2 = 128 * (ci - c)
                if w < 128:
                    # partial chunk: clear the never-written psum rows so
                    # no stale inf/NaN reaches the fp8 h tiles (32-aligned
                    # partition base; transpose rewrites the valid overlap)
                    base = (w // 32) * 32
                    nc.vector.memset(pt[base:128, fo2:fo2 + 128], 0.0)
                nc.tensor.transpose(pt[0:w, fo2:fo2 + 128],
                                    xs[:, 128 * ci:128 * ci + w],
                                    ident[:, :])
            # split copies into the stream tiles (3D strided APs)
            nb = cn - c
            for s in range(N_STR):
                src = pt[:, W_S * s:W_S * s + (nb - 1) * 128 + W_S]
                sv = src.copy()
                pstep = sv.ap[0][0]
                sv.ap = type(sv.ap)([[pstep, 128], [128, nb], [1, W_S]])
                dst = h0[s][:, W_S * c: W_S * cn]
                dv = dst.copy()
                pstep2 = dv.ap[0][0]
                dv.ap = type(dv.ap)([[pstep2, 128], [W_S, nb], [1, W_S]])
                bal.copy(dv, sv, nb * W_S, 0.0, 0.0, False)
            c = cn

        # ---- 200 conv layers: one matmul + one copy per stream ----
        hin = h0
        for j in range(1, N_LAYERS + 1):
            Lo = L0 - 6 * j
            nco = _ceil_div(Lo, 128)
            N = nco * W_S
            lw = bchunks[(j - 1) // WCHUNK_LAYERS]
            fo = 256 * ((j - 1) % WCHUNK_LAYERS)
            lhsT = lw[:, fo:fo + 256].rearrange("p (two m) -> p two m", two=2)

            bias_ap = cb_s[:, j - 1:j]
            bias_val = float(conv_b[j - 1])
            relu = j < N_LAYERS

            hout = [hpool.tile([128, (nco + 1) * W_S], F8, tag=f"h{s}",
                               name=f"h{j}_{s}")
                    for s in range(N_STR)]
            if j <= 2:
                # first tenancy of pool buffers: clear virgin SBUF so the
                # pad/garbage regions stay finite fp8 forever
                for s in range(N_STR):
                    nc.gpsimd.memset(hout[s][:, :], 0.0)

            pss = []
            for s in range(N_STR):
                ps = pspool.tile([128, N], F32, tag=f"ps{s}",
                                 name=f"ps{j}_{s}")
                rhs = _dr_view(hin[s][:, 0:N + W_S], W_S, N)
                nc.tensor.matmul(ps[:, 0:N], lhsT, rhs,
                                 start=True, stop=True, perf_mode=DR,
                                 skip_group_check=True)
                pss.append(ps)

            for s in range(N_STR):
                bal.copy(hout[s][:, 0:N], pss[s][:, 0:N],
                         N, bias_ap, bias_val, relu)

            hin = hout

        # ---- FC 188 -> 91 (DoubleRow over the two position chunks) ----
        fclhs = fcp[:, 0:192].rearrange("p (two m) -> p two m", two=2)
        fps = pspool.tile([96, 128], F32, tag="ps0")
        for s in range(N_STR):
            rhs = _dr_view(hin[s][:, 0:2 * W_S], W_S, W_S)
            nc.tensor.matmul(fps[:, W_S * s:W_S * s + W_S], fclhs, rhs,
                             start=True, stop=True, perf_mode=DR,
                             skip_group_check=True)
        nc.scalar.activation(outs[:, :], fps[0:FC_OUT, :], AFT.Sigmoid,
                             bias=fcbias[0:FC_OUT, 0:1], scale=1.0)
        nc.sync.dma_start(out_d[:, :], outs[:, :])

    nc.compile()
    return nc


def make_in_maps(x, conv_w, conv_b, fc_w, fc_b):
    x = np.ascontiguousarray(np.asarray(x, np.float32))
    bands = np.ascontiguousarray(
        _make_bands(np.asarray(conv_w, np.float32)).astype(E4M3))
    cb = np.ascontiguousarray(
        np.broadcast_to(np.asarray(conv_b, np.float32)[None, :],
                        (128, N_LAYERS)))
    fcpack = np.ascontiguousarray(_make_fcpack(fc_w).astype(E4M3))
    fcb = np.ascontiguousarray(np.asarray(fc_b, np.float32)[:, None])
    ident = np.eye(128, dtype=np.float32)
    in_maps = []
    for i in range(N_CORES):
        in_maps.append({
            "xs": np.ascontiguousarray(x[BC * i:BC * (i + 1)]),
            "bands": bands,
            "cb": cb,
            "fcpack": fcpack,
            "fcb": fcb,
            "ident": ident,
        })
    return in_maps


def _enable_ldw_dedup():
    """Re-enable walrus's LDWEIGHTS dedup pass for this process: our conv
    layers issue 2-4 matmuls per stationary, so the per-matmul reload is
    pure overhead."""
    import concourse.bass_utils as bu
    if getattr(bu.run_command, "_ldw_patched", False):
        return
    orig = bu.run_command

    def patched(argv, **kw):
        argv = ["--enable-ldw-opt=true" if a == "--enable-ldw-opt=false"
                else a for a in argv]
        return orig(argv, **kw)

    patched._ldw_patched = True
    bu.run_command = patched


def run(x, conv_w, conv_b, fc_w, fc_b, **spmd_kwargs):
    nc = build_program(np.asarray(conv_b, np.float32))
    in_maps = make_in_maps(x, conv_w, conv_b, fc_w, fc_b)
    res = run_bass_kernel_spmd(nc, in_maps, list(range(N_CORES)),
                               **spmd_kwargs)
    out = np.concatenate([r["outT"].T for r in res.results], axis=0)
    return np.ascontiguousarray(out, np.float32), res


def kernel(x, conv_w, conv_b, fc_w, fc_b):
    out, _ = run(x, conv_w, conv_b, fc_w, fc_b)
    return out


if __name__ == "__main__":
    rng = np.random.default_rng(0)
    x = rng.normal(size=(1024, L0)).astype(np.float32)
    s = 1.0 / np.sqrt(7)
    cw = rng.uniform(-s, s, (N_LAYERS, 7)).astype(np.float32)
    cb = rng.uniform(-s, s, N_LAYERS).astype(np.float32)
    sf = 1.0 / np.sqrt(FC_IN)
    fw = rng.uniform(-sf, sf, (FC_OUT, FC_IN)).astype(np.float32)
    fb = rng.uniform(-sf, sf, FC_OUT).astype(np.float32)
    out = kernel(x, cw, cb, fw, fb)
    print(out.shape, out.dtype)
